# revision 41
# baseline (speedup 1.0000x reference)
"""AsymAttentionLayer Trainium2 kernel — data-parallel over B on 8 NeuronCores.

Reference computation (per batch element b, NUM_G=32, g=32, num_p=8, dim=512,
H=8, E=64):
  stage1: attention within groups of g=32 tokens (seq=(ng,p), pos=gg)
  stage2: attention across groups (seq=(gg,p), pos=ng)
  7 linears of [tokens,512]x[512,512].

v2: the six q/k/v linears run in fp8e4m3 with MatmulPerfMode.DoubleRow
(256-deep contraction per pass, 2x the bf16 column rate); the out linear
stays bf16 (its quantization error alone would exceed the error budget).
All v-biases are folded into downstream biases on the host (softmax rows
sum to 1, so A@(V+1 b^T) = A@V + 1 b^T), which turns every attention-output
evacuation into a pure cast that can run on any engine.  Attention math is
unchanged from the baseline (bf16, PE-quadrant-packed 32x32 blocks).

Device-side layouts: fp8 activations/weights use the DoubleRow pairing
[128 part, (i in 2, col)] where partition p + pair i covers input feature
256*j + 128*i + p for pass j.  Token order t = ng*256 + p*32 + gg.
"""

import os
import sys

import numpy as np

sys.path.insert(0, "/opt/trn_rl_repo")

NUM_G = 32
G = 32
NUM_P = 8
B = 8
D = 512
H = 8
E = 64
T = NUM_G * G * NUM_P  # 8192 tokens per core
CH = 512  # tokens per chunk
NCHUNK = T // CH  # 16
NSEQ = CH // G  # 16 sequences per chunk

_GRAPH = None
LAST_EXEC_TIME_NS = None
LAST_TRACE = None


def _build_graph():
    import concourse.bass as bass
    from concourse import bacc, mybir
    from concourse.tile import TileContext

    f32 = mybir.dt.float32
    bf16 = mybir.dt.bfloat16
    e4 = mybir.dt.float8e4
    DR = mybir.MatmulPerfMode.DoubleRow
    AF = mybir.ActivationFunctionType
    ALU = mybir.AluOpType
    AX = mybir.AxisListType

    nc = bacc.Bacc()

    # fp8 activations, DoubleRow-paired: x8[p, 2j+i, t] = x[t, 256j+128i+p]
    x8_d = nc.declare_dram_parameter("x8", [128, 4, T], e4, isOutput=False)
    w8_d = {}
    for name in ("wq1", "wk1", "wv1", "wq2", "wk2", "wv2"):
        # w8[p, 2j+i, o] = W'[o, 256j+128i+p]
        w8_d[name] = nc.declare_dram_parameter(name, [128, 4, D], e4, isOutput=False)
    wo_d = nc.declare_dram_parameter("wo", [D, D], bf16, isOutput=False)
    b_d = {}
    for name in ("bq1", "bq2", "bo"):
        # k-biases are dropped entirely: softmax over s is invariant to the
        # s-independent terms q_l.bk + bq.bk, so S' = (q+bq).k is exact.
        b_d[name] = nc.declare_dram_parameter(name, [128, 4], f32, isOutput=False)
    out_d = nc.declare_dram_parameter("out", [D, T], bf16, isOutput=True)

    with TileContext(nc) as tc:
        with (
            tc.tile_pool(name="wpool", bufs=1) as wpool,
            tc.tile_pool(name="bpool", bufs=1) as bpool,
            tc.tile_pool(name="y1pool", bufs=1) as y1pool,
            tc.tile_pool(name="sbx", bufs=4) as sbx,
            tc.tile_pool(name="sbqk", bufs=2) as sbqk,
            tc.tile_pool(name="sbv", bufs=2) as sbv,
            tc.tile_pool(name="sba", bufs=2) as sba,
            tc.tile_pool(name="sbo", bufs=2) as sbo,
            tc.tile_pool(name="pp", bufs=8, space="PSUM") as pp,
        ):
            # ---- weights / biases resident in SBUF ----
            wt8 = {}
            wo_t = []
            bt = {}

            def load_w8(name):
                t_ = wpool.tile([128, 4 * D], e4, tag=f"w8_{name}", name=f"w8_{name}")
                nc.sync.dma_start(
                    out=t_.rearrange("p (a f) -> p a f", a=4), in_=w8_d[name][:, :, :]
                )
                # [p, j, i, f]
                wt8[name] = t_.rearrange("p (j i f) -> p j i f", j=2, i=2)

            def load_wo():
                for k in range(4):
                    t_ = wpool.tile([128, D], bf16, tag=f"wo_{k}", name=f"wo_{k}")
                    nc.sync.dma_start(out=t_, in_=wo_d[128 * k : 128 * (k + 1), :])
                    wo_t.append(t_)

            def load_b(name):
                t_ = bpool.tile([128, 4], f32, tag=name, name=f"b_{name}")
                nc.sync.dma_start(out=t_, in_=b_d[name][:, :])
                bt[name] = t_

            for name in ("wq1", "wk1", "wv1"):
                load_w8(name)
            load_b("bq1")

            # stage-1 output, fp8 DoubleRow-paired: y1_8[j] cols = i*T + t
            y1_8 = [
                y1pool.tile([128, 2 * T], e4, tag=f"y18_{j}", name=f"y18_{j}")
                for j in range(2)
            ]
            y1r = [y1_8[j].rearrange("p (i t) -> p i t", i=2) for j in range(2)]
            # scatter view for av_evac with merged (i,g) dim:
            # col = (i*32+g)*256 + q4*128 + r*32 + ng
            y1s = [
                y1_8[j].rearrange(
                    "p (ig q4 qr n) -> p qr n q4 ig", ig=64, q4=2, qr=4, n=32
                )
                for j in range(2)
            ]

            def linear_fp8(w8r, bias_tile, rhs8, out_tiles):
                """Feature-major fp8 DoubleRow linear: out[m] = W'^T@x (+ b).

                rhs8: list of 2 APs [128, 2, 512] (pass j in {0,1}).
                out_tiles: 4 SBUF tiles [128, 512] bf16.
                Evacuations alternate ACT/DVE so two drains run in parallel
                with the PE's psum production.
                """
                for m in range(4):
                    ps = pp.tile([128, CH], f32, tag="ps", name="ps_lin")
                    for j in range(2):
                        nc.tensor.matmul(
                            ps,
                            lhsT=w8r[:, j, :, 128 * m : 128 * (m + 1)],
                            rhs=rhs8[j],
                            start=(j == 0),
                            stop=(j == 1),
                            perf_mode=DR,
                        )
                    if bias_tile is None:
                        if m % 2 == 0:
                            nc.vector.tensor_copy(out_tiles[m], ps)
                        else:
                            nc.scalar.copy(out_tiles[m], ps)
                    elif m % 2 == 0:
                        nc.scalar.activation(
                            out=out_tiles[m],
                            in_=ps,
                            func=AF.Identity,
                            bias=bias_tile[:, m : m + 1],
                        )
                    else:
                        nc.vector.tensor_scalar_add(
                            out_tiles[m], ps, bias_tile[:, m : m + 1]
                        )

            def linear_fp8_token_major(w8r, lhsT8, out_tiles):
                """v-linear: out[j] = [128 tokens, 512 dims], no bias (folded).

                lhsT8[jj][j]: AP [128, 2, 128] - token group jj, pass j.
                """
                for jj in range(4):
                    ps = pp.tile([128, D], f32, tag="ps", name="ps_vlin")
                    for j in range(2):
                        nc.tensor.matmul(
                            ps,
                            lhsT=lhsT8[jj][j],
                            rhs=w8r[:, j],
                            start=(j == 0),
                            stop=(j == 1),
                            perf_mode=DR,
                        )
                    if jj % 2 == 0:
                        nc.vector.tensor_copy(out_tiles[jj], ps)
                    else:
                        nc.scalar.copy(out_tiles[jj], ps)

            def scores_begin():
                a_f = [sba.tile([128, CH], bf16, tag=f"a{i}", name=f"a_f{i}") for i in range(2)]
                sums = sba.tile([128, 32], f32, tag="sums")
                a_n = [sba.tile([128, CH], bf16, tag=f"an{i}", name=f"a_n{i}") for i in range(2)]
                a_t = [sba.tile([128, CH], bf16, tag=f"at{i}", name=f"a_t{i}") for i in range(2)]
                rs = sba.tile([128, 32], f32, tag="rs")
                return dict(a_f=a_f, sums=sums, a_n=a_n, a_t=a_t, rs=rs)

            def scores_half(qt, kt, sb, sst):
                """One head-parity's scores + softmax (bank sb).  Emitted in
                two halves so the fp8 linear bursts interleave with the
                low-duty attention matmuls (power smoothing + psum slack).

                PSUM bank index always equals the PE row-tile index so that
                concurrently-running row tiles never write the same bank.
                scores placement: bank=h%2 (= row tile 64*(h%2)), strip=sl%4,
                colblk=(h//2)*4 + sl//4.
                """
                a_f, sums, a_n, a_t, rs = (
                    sst["a_f"], sst["sums"], sst["a_n"], sst["a_t"], sst["rs"]
                )
                if True:
                    ps_sb = pp.tile([128, CH], f32, tag="ps", name="ps_s")
                    for sl in range(NSEQ):
                        st = 32 * (sl % 4)
                        for h in range(sb, H, 2):
                            cb = 32 * ((h // 2) * 4 + sl // 4)
                            rb = 64 * sb
                            nc.tensor.matmul(
                                ps_sb[st : st + 32, cb : cb + 32],
                                lhsT=qt[h // 2][rb : rb + 64, 32 * sl : 32 * sl + 32],
                                rhs=kt[h // 2][rb : rb + 64, 32 * sl : 32 * sl + 32],
                                start=True,
                                stop=True,
                                tile_position=(rb, st),
                            )
                    # softmax over s (free dim), batched 64 problems per op
                    nc.scalar.activation(out=a_f[sb], in_=ps_sb, func=AF.Exp)
                    nc.vector.tensor_reduce(
                        out=sums[:, 16 * sb : 16 * sb + 16],
                        in_=a_f[sb].rearrange("p (j s) -> p j s", s=32),
                        axis=AX.X,
                        op=ALU.add,
                    )
                    rs_sl = rs[:, 16 * sb : 16 * sb + 16]
                    nc.vector.reciprocal(rs_sl, sums[:, 16 * sb : 16 * sb + 16])
                    rs_b = bass.AP(
                        tensor=rs_sl.tensor,
                        offset=rs_sl.offset,
                        ap=[*rs_sl.ap, [0, 32]],
                    )
                    nc.gpsimd.tensor_mul(
                        a_n[sb].rearrange("p (j s) -> p j s", s=32),
                        a_f[sb].rearrange("p (j s) -> p j s", s=32),
                        rs_b,
                    )
                    nc.vector.transpose(a_t[sb], a_n[sb])

            def av_group(vt, a_t, evac_fn, grp):
                """One bank-pair of AV matmuls + evacuation (pure casts).

                psum bank r = row tile 32r holds the 32 problems with
                sl%4 == r; partition rows 64*(h%2) = head parity.  The a_t
                input block sits at the scores layout column
                cb_s = 32*((h//2)*4 + sl//4); the OUTPUT goes to column
                cc = 8j + 4a + 2q4 + i (hh = h//2 = 2j+i, sl//4 = 2a+q4) so
                that (i,g) merges into one AP dim for the fp8 evacuation.
                Banks {0,1} fill and drain before {2,3} allocate, smoothing
                psum-ring pressure.
                """
                ys = [pp.tile([128, CH], f32, tag="ps", name="ps_y") for _ in range(2)]
                for sb in range(2):
                    for sl in range(NSEQ):
                        if (sl % 4) // 2 != grp:
                            continue
                        st = 32 * (sl % 4)
                        s4 = sl // 4
                        a, q4 = s4 // 2, s4 % 2
                        for h in range(sb, H, 2):
                            cb_s = 32 * ((h // 2) * 4 + s4)
                            hh = h // 2
                            cc = 8 * (hh // 2) + 4 * a + 2 * q4 + (hh % 2)
                            nc.tensor.matmul(
                                ys[(sl % 4) % 2][
                                    64 * sb : 64 * sb + 64, 32 * cc : 32 * cc + 32
                                ],
                                lhsT=vt[sl // 4][st : st + 32, 64 * h : 64 * h + 64],
                                rhs=a_t[sb][st : st + 32, cb_s : cb_s + 32],
                                start=True,
                                stop=True,
                                tile_position=(st, 64 * sb),
                            )
                for rr in range(2):
                    evac_fn(2 * grp + rr, ys[rr])

            def produce_lin(i):
                """Linears for pipeline step i (i<16: stage-1 chunk i,
                i>=16: stage-2 chunk i-16).  Returns the attention inputs."""
                qt = [sbqk.tile([128, CH], bf16, tag=f"qt{m}", name=f"qt{m}") for m in range(4)]
                kt = [sbqk.tile([128, CH], bf16, tag=f"kt{m}", name=f"kt{m}") for m in range(4)]
                vt = [sbv.tile([128, D], bf16, tag=f"vt{j}", name=f"vt{j}") for j in range(4)]
                if i < NCHUNK:
                    c = i
                    cols = slice(CH * c, CH * (c + 1))
                    xc8 = []
                    for j in range(2):
                        t_ = sbx.tile([128, 2 * CH], e4, tag=f"xc8_{j}", name=f"xc8_{j}")
                        nc.sync.dma_start(
                            out=t_.rearrange("p (i t) -> p i t", i=2),
                            in_=x8_d[:, 2 * j : 2 * j + 2, cols],
                        )
                        xc8.append(t_.rearrange("p (i t) -> p i t", i=2))
                    rhs8 = [xc8[j][:, :, :] for j in range(2)]
                    lhsT8 = [
                        [xc8[j][:, :, 128 * jj : 128 * (jj + 1)] for j in range(2)]
                        for jj in range(4)
                    ]
                    eq = lambda: linear_fp8(wt8["wq1"], bt["bq1"], rhs8, qt)
                    ek = lambda: linear_fp8(wt8["wk1"], None, rhs8, kt)
                    ev = lambda: linear_fp8_token_major(wt8["wv1"], lhsT8, vt)

                    def evac_fn(r, ys_r, c=c):
                        # 2 ops per bank: pass j -> [p, a, q4, (i,g)]
                        for j in range(2):
                            src = ys_r[:, 256 * j : 256 * j + 256].rearrange(
                                "p (a q ig) -> p a q ig", a=2, q=2
                            )
                            dst = y1s[j][:, r, 2 * c : 2 * c + 2, :, :]
                            if (r + j) % 2:
                                nc.scalar.copy(dst, src)
                            else:
                                nc.vector.tensor_copy(dst, src)

                    return (eq, ek, ev), dict(qt=qt, kt=kt, vt=vt, evac=evac_fn, y2t=None)
                c2 = i - NCHUNK
                cols = slice(CH * c2, CH * (c2 + 1))
                rhs8 = [y1r[j][:, :, cols] for j in range(2)]
                lhsT8 = [
                    [
                        y1r[j][:, :, CH * c2 + 128 * jj : CH * c2 + 128 * (jj + 1)]
                        for j in range(2)
                    ]
                    for jj in range(4)
                ]
                eq = lambda: linear_fp8(wt8["wq2"], bt["bq2"], rhs8, qt)
                ek = lambda: linear_fp8(wt8["wk2"], None, rhs8, kt)
                ev = lambda: linear_fp8_token_major(wt8["wv2"], lhsT8, vt)
                y2t = [sbo.tile([128, CH], bf16, tag=f"y2_{r}", name=f"y2_{r}") for r in range(4)]
                y2x = [
                    y2t[hh].rearrange(
                        "p (a q4 qr g) -> p qr a q4 g", a=2, q4=2, qr=4, g=32
                    )
                    for hh in range(4)
                ]

                def evac_fn(r, ys_r):
                    ysr = ys_r.rearrange(
                        "p (jj a q4 ii g) -> p jj ii a q4 g", jj=2, a=2, q4=2, ii=2
                    )
                    for hh in range(4):
                        src = ysr[:, hh // 2, hh % 2]
                        dst = y2x[hh][:, r, :, :, :]
                        if (r + hh) % 2 == 0:
                            nc.vector.tensor_copy(dst, src)
                        else:
                            nc.scalar.copy(dst, src)

                return (eq, ek, ev), dict(qt=qt, kt=kt, vt=vt, evac=evac_fn, y2t=y2t)

            def out_linear(c2, y2t, ms=(0, 1, 2, 3)):
                for m in ms:
                    ps = pp.tile([128, CH], f32, tag="ps", name="ps_lin")
                    for k in range(4):
                        nc.tensor.matmul(
                            ps,
                            lhsT=wo_t[k][:, 128 * m : 128 * (m + 1)],
                            rhs=y2t[k],
                            start=(k == 0),
                            stop=(k == 3),
                        )
                    os_ = sbo.tile([128, CH], bf16, tag=f"os{m}", name=f"os{m}")
                    if m % 2 == 0:
                        nc.scalar.activation(
                            out=os_, in_=ps, func=AF.Identity,
                            bias=bt["bo"][:, m : m + 1],
                        )
                    else:
                        nc.vector.tensor_scalar_add(os_, ps, bt["bo"][:, m : m + 1])
                    nc.sync.dma_start(
                        out=out_d[128 * m : 128 * (m + 1), CH * c2 : CH * (c2 + 1)],
                        in_=os_,
                    )

            # ---- software pipeline over 32 steps (16 per stage): the PE
            # stream per step interleaves the low-duty attention matmuls
            # with the dense fp8 bursts: [scores-half0(i), q-lin(i+1),
            # scores-half1(i), k-lin(i+1), outlin(i-1), v-lin(i+1), AV(i)]
            # — power smoothing plus extra psum drain slack.  Exception:
            # stage-2 linears read y1, so produce_lin(16) comes after
            # av_evac(15).
            NSTEP = 2 * NCHUNK
            emits, st = produce_lin(0)
            for e in emits:
                e()
            # remaining (stage-2 / out) weights load behind the critical path
            for name in ("wq2", "wk2", "wv2"):
                load_w8(name)
            load_wo()
            for name in ("bq2", "bo"):
                load_b(name)
            pend_out = None
            for i in range(NSTEP):
                sst = scores_begin()
                scores_half(st["qt"], st["kt"], 0, sst)
                if i + 1 < NSTEP and i != NCHUNK - 1:
                    (eq, ek, ev), st_next = produce_lin(i + 1)
                    eq()
                    scores_half(st["qt"], st["kt"], 1, sst)
                    ek()
                    av_group(st["vt"], sst["a_t"], st["evac"], 0)
                    if pend_out is not None:
                        out_linear(*pend_out, ms=(0, 1))
                    av_group(st["vt"], sst["a_t"], st["evac"], 1)
                    ev()
                    if pend_out is not None:
                        out_linear(*pend_out, ms=(2, 3))
                        pend_out = None
                else:
                    st_next = None
                    scores_half(st["qt"], st["kt"], 1, sst)
                    av_group(st["vt"], sst["a_t"], st["evac"], 0)
                    if pend_out is not None:
                        out_linear(*pend_out, ms=(0, 1))
                    av_group(st["vt"], sst["a_t"], st["evac"], 1)
                    if pend_out is not None:
                        out_linear(*pend_out, ms=(2, 3))
                        pend_out = None
                if i == NCHUNK - 1:
                    (eq, ek, ev), st_next = produce_lin(i + 1)
                    eq()
                    ek()
                    ev()
                if st["y2t"] is not None:
                    pend_out = (i - NCHUNK, st["y2t"])
                st = st_next
            out_linear(*pend_out)
    nc.finalize()
    return nc


def _get_graph():
    global _GRAPH
    if _GRAPH is None:
        _GRAPH = _build_graph()
    return _GRAPH


def _host_pack(x, q1_w, q1_b, k1_w, k1_b, v1_w, v1_b, q2_w, q2_b, k2_w, k2_b,
               v2_w, v2_b, out_w, out_b):
    import ml_dtypes

    bf = ml_dtypes.bfloat16
    e4 = ml_dtypes.float8_e4m3
    scale = 1.0 / np.sqrt(E)

    def w8(w, s=1.0):
        # [128, 4, D]: w8[p, 2j+i, o] = (W*s)[o, 256j+128i+p]
        wT = (w * s).astype(np.float32).T.reshape(2, 2, 128, D)  # [j, i, p, o]
        return np.ascontiguousarray(wT.transpose(2, 0, 1, 3).reshape(128, 4, D)).astype(e4)

    def bia(b, s=1.0):
        return np.ascontiguousarray(
            (np.asarray(b, dtype=np.float64) * s).astype(np.float32).reshape(4, 128).T
        )

    # fold v-biases into downstream biases (softmax rows sum to 1):
    #   y1_true = y1_nb + 1 v1_b^T  =>  q2/v2 biases pick up W @ v1_b
    #   y2_true = y2_nb + 1 c2^T, c2 = v2_b + v2_w@v1_b  =>  out bias += out_w@c2
    # k-biases are dropped (softmax shift invariance).
    q1_bf = np.asarray(q1_b, dtype=np.float64)
    v1_bf = np.asarray(v1_b, dtype=np.float64)
    q2_be = np.asarray(q2_b, dtype=np.float64) + np.asarray(q2_w, dtype=np.float64) @ v1_bf
    c2 = np.asarray(v2_b, dtype=np.float64) + np.asarray(v2_w, dtype=np.float64) @ v1_bf
    bo_e = np.asarray(out_b, dtype=np.float64) + np.asarray(out_w, dtype=np.float64) @ c2

    common = {
        "wq1": w8(q1_w, scale), "wk1": w8(k1_w), "wv1": w8(v1_w),
        "wq2": w8(q2_w, scale), "wk2": w8(k2_w), "wv2": w8(v2_w),
        "wo": np.ascontiguousarray(np.asarray(out_w, dtype=np.float32).T).astype(bf),
        "bq1": bia(q1_bf, scale),
        "bq2": bia(q2_be, scale),
        "bo": bia(bo_e),
    }
    in_maps = []
    for b in range(B):
        # x[b]: [1024(ch=ng*32+gg), 8(p), 512] -> token order t = ng*256+p*32+gg
        xb = np.asarray(x[b]).reshape(NUM_G, G, NUM_P, D)
        xb = xb.transpose(0, 2, 1, 3).reshape(T, D)
        # x8[p, 2j+i, t] = xb[t, 256j+128i+p]
        x8 = xb.T.reshape(2, 2, 128, T).transpose(2, 0, 1, 3).reshape(128, 4, T)
        m = dict(common)
        m["x8"] = np.ascontiguousarray(x8).astype(e4)
        in_maps.append(m)
    return in_maps


def _host_unpack(results):
    # device out: [512, 8192] f32, cols packed:
    # col(ng,gg,p) = (gg//2)*512 + (gg%2)*256 + p*32 + ng
    ng_, gg_, p_ = np.meshgrid(
        np.arange(NUM_G), np.arange(G), np.arange(NUM_P), indexing="ij"
    )
    idx = (gg_ // 2) * 512 + (gg_ % 2) * 256 + p_ * 32 + ng_
    out = np.empty((B, NUM_G * G, NUM_P, D), dtype=np.float32)
    for b in range(B):
        y = results[b]["out"].T.astype(np.float32)  # [8192, 512]
        out[b] = y[idx].reshape(NUM_G * G, NUM_P, D)
    return out


def kernel(**inputs):
    global LAST_EXEC_TIME_NS, LAST_TRACE
    from concourse.bass_utils import run_bass_kernel_spmd

    nc = _get_graph()
    in_maps = _host_pack(**inputs)
    trace = os.environ.get("KBENCH_TRACE") == "1"
    res = run_bass_kernel_spmd(nc, in_maps, list(range(8)), trace=trace)
    LAST_EXEC_TIME_NS = res.exec_time_ns
    it = res.instructions_and_trace
    LAST_TRACE = it[1] if it else None
    return _host_unpack(res.results)


# revision 42
# speedup vs baseline: 1.1055x; 1.1055x over previous
"""AsymAttentionLayer Trainium2 kernel — data-parallel over B on 8 NeuronCores.

Reference computation (per batch element b, NUM_G=32, g=32, num_p=8, dim=512,
H=8, E=64):
  stage1: attention within groups of g=32 tokens (seq=(ng,p), pos=gg)
  stage2: attention across groups (seq=(gg,p), pos=ng)
  7 linears of [tokens,512]x[512,512].

v2: the six q/k/v linears run in fp8e4m3 with MatmulPerfMode.DoubleRow
(256-deep contraction per pass, 2x the bf16 column rate); the out linear
stays bf16 (its quantization error alone would exceed the error budget).
All v-biases are folded into downstream biases on the host (softmax rows
sum to 1, so A@(V+1 b^T) = A@V + 1 b^T), which turns every attention-output
evacuation into a pure cast that can run on any engine.  Attention math is
unchanged from the baseline (bf16, PE-quadrant-packed 32x32 blocks).

Device-side layouts: fp8 activations/weights use the DoubleRow pairing
[128 part, (i in 2, col)] where partition p + pair i covers input feature
256*j + 128*i + p for pass j.  Token order t = ng*256 + p*32 + gg.
"""

import os
import sys

import numpy as np

sys.path.insert(0, "/opt/trn_rl_repo")

NUM_G = 32
G = 32
NUM_P = 8
B = 8
D = 512
H = 8
E = 64
T = NUM_G * G * NUM_P  # 8192 tokens per core
CH = 512  # tokens per chunk
NCHUNK = T // CH  # 16
NSEQ = CH // G  # 16 sequences per chunk

_GRAPH = None
LAST_EXEC_TIME_NS = None
LAST_TRACE = None


def _build_graph():
    import concourse.bass as bass
    from concourse import bacc, mybir
    from concourse.tile import TileContext

    f32 = mybir.dt.float32
    bf16 = mybir.dt.bfloat16
    e4 = mybir.dt.float8e4
    DR = mybir.MatmulPerfMode.DoubleRow
    AF = mybir.ActivationFunctionType
    ALU = mybir.AluOpType
    AX = mybir.AxisListType

    nc = bacc.Bacc()

    # fp8 activations, DoubleRow-paired: x8[p, 2j+i, t] = x[t, 256j+128i+p]
    x8_d = nc.declare_dram_parameter("x8", [128, 4, T], e4, isOutput=False)
    w8_d = {}
    for name in ("wq1", "wk1", "wv1", "wq2", "wk2", "wv2"):
        # w8[p, 2j+i, o] = W'[o, 256j+128i+p]
        w8_d[name] = nc.declare_dram_parameter(name, [128, 4, D], e4, isOutput=False)
    wo_d = nc.declare_dram_parameter("wo", [D, D], bf16, isOutput=False)
    b_d = {}
    for name in ("bq1", "bq2", "bo"):
        # k-biases are dropped entirely: softmax over s is invariant to the
        # s-independent terms q_l.bk + bq.bk, so S' = (q+bq).k is exact.
        b_d[name] = nc.declare_dram_parameter(name, [128, 4], f32, isOutput=False)
    out_d = nc.declare_dram_parameter("out", [D, T], bf16, isOutput=True)

    with TileContext(nc) as tc:
        with (
            tc.tile_pool(name="wpool", bufs=1) as wpool,
            tc.tile_pool(name="bpool", bufs=1) as bpool,
            tc.tile_pool(name="y1pool", bufs=1) as y1pool,
            tc.tile_pool(name="sbx", bufs=4) as sbx,
            tc.tile_pool(name="sbqk", bufs=2) as sbqk,
            tc.tile_pool(name="sbv", bufs=2) as sbv,
            tc.tile_pool(name="sba", bufs=2) as sba,
            tc.tile_pool(name="sbo", bufs=2) as sbo,
            tc.tile_pool(name="pp", bufs=8, space="PSUM") as pp,
        ):
            # ---- weights / biases resident in SBUF ----
            wt8 = {}
            wo_t = []
            bt = {}

            def load_w8(name):
                t_ = wpool.tile([128, 4 * D], e4, tag=f"w8_{name}", name=f"w8_{name}")
                nc.sync.dma_start(
                    out=t_.rearrange("p (a f) -> p a f", a=4), in_=w8_d[name][:, :, :]
                )
                # [p, j, i, f]
                wt8[name] = t_.rearrange("p (j i f) -> p j i f", j=2, i=2)

            def load_wo():
                for k in range(4):
                    t_ = wpool.tile([128, D], bf16, tag=f"wo_{k}", name=f"wo_{k}")
                    nc.sync.dma_start(out=t_, in_=wo_d[128 * k : 128 * (k + 1), :])
                    wo_t.append(t_)

            def load_b(name):
                t_ = bpool.tile([128, 4], f32, tag=name, name=f"b_{name}")
                nc.sync.dma_start(out=t_, in_=b_d[name][:, :])
                bt[name] = t_

            for name in ("wq1", "wk1", "wv1"):
                load_w8(name)
            load_b("bq1")

            # stage-1 output, fp8 DoubleRow-paired: y1_8[j] cols = i*T + t
            y1_8 = [
                y1pool.tile([128, 2 * T], e4, tag=f"y18_{j}", name=f"y18_{j}")
                for j in range(2)
            ]
            y1r = [y1_8[j].rearrange("p (i t) -> p i t", i=2) for j in range(2)]
            # scatter view for av_evac with merged (i,g) dim:
            # col = (i*32+g)*256 + q4*128 + r*32 + ng
            y1s = [
                y1_8[j].rearrange(
                    "p (ig q4 qr n) -> p qr n q4 ig", ig=64, q4=2, qr=4, n=32
                )
                for j in range(2)
            ]

            def linear_fp8(w8r, bias_tile, rhs8, out_tiles):
                """Feature-major fp8 DoubleRow linear: out[m] = W'^T@x (+ b).

                rhs8: list of 2 APs [128, 2, 512] (pass j in {0,1}).
                out_tiles: 4 SBUF tiles [128, 512] bf16.
                Evacuations alternate ACT/DVE so two drains run in parallel
                with the PE's psum production.
                """
                for m in range(4):
                    ps = pp.tile([128, CH], f32, tag="ps", name="ps_lin")
                    for j in range(2):
                        nc.tensor.matmul(
                            ps,
                            lhsT=w8r[:, j, :, 128 * m : 128 * (m + 1)],
                            rhs=rhs8[j],
                            start=(j == 0),
                            stop=(j == 1),
                            perf_mode=DR,
                        )
                    if bias_tile is None:
                        if m % 2 == 0:
                            nc.vector.tensor_copy(out_tiles[m], ps)
                        else:
                            nc.scalar.copy(out_tiles[m], ps)
                    elif m % 2 == 0:
                        nc.scalar.activation(
                            out=out_tiles[m],
                            in_=ps,
                            func=AF.Identity,
                            bias=bias_tile[:, m : m + 1],
                        )
                    else:
                        nc.vector.tensor_scalar_add(
                            out_tiles[m], ps, bias_tile[:, m : m + 1]
                        )

            def linear_fp8_token_major(w8r, lhsT8, out_tiles):
                """v-linear: out[j] = [128 tokens, 512 dims], no bias (folded).

                lhsT8[jj][j]: AP [128, 2, 128] - token group jj, pass j.
                """
                for jj in range(4):
                    ps = pp.tile([128, D], f32, tag="ps", name="ps_vlin")
                    for j in range(2):
                        nc.tensor.matmul(
                            ps,
                            lhsT=lhsT8[jj][j],
                            rhs=w8r[:, j],
                            start=(j == 0),
                            stop=(j == 1),
                            perf_mode=DR,
                        )
                    if jj % 2 == 0:
                        nc.vector.tensor_copy(out_tiles[jj], ps)
                    else:
                        nc.scalar.copy(out_tiles[jj], ps)

            def scores_begin():
                a_f = [sba.tile([128, CH], bf16, tag=f"a{i}", name=f"a_f{i}") for i in range(2)]
                sums = sba.tile([128, 32], f32, tag="sums")
                a_n = [sba.tile([128, CH], bf16, tag=f"an{i}", name=f"a_n{i}") for i in range(2)]
                a_t = [sba.tile([128, CH], bf16, tag=f"at{i}", name=f"a_t{i}") for i in range(2)]
                rs = sba.tile([128, 32], f32, tag="rs")
                return dict(a_f=a_f, sums=sums, a_n=a_n, a_t=a_t, rs=rs)

            def scores_half(qt, kt, sb, sst):
                """One head-parity's scores + softmax (bank sb).  Emitted in
                two halves so the fp8 linear bursts interleave with the
                low-duty attention matmuls (power smoothing + psum slack).

                PSUM bank index always equals the PE row-tile index so that
                concurrently-running row tiles never write the same bank.
                scores placement: bank=h%2 (= row tile 64*(h%2)), strip=sl%4,
                colblk=(h//2)*4 + sl//4.
                """
                a_f, sums, a_n, a_t, rs = (
                    sst["a_f"], sst["sums"], sst["a_n"], sst["a_t"], sst["rs"]
                )
                if True:
                    ps_sb = pp.tile([128, CH], f32, tag="ps", name="ps_s")
                    for sl in range(NSEQ):
                        st = 32 * (sl % 4)
                        for h in range(sb, H, 2):
                            cb = 32 * ((h // 2) * 4 + sl // 4)
                            rb = 64 * sb
                            nc.tensor.matmul(
                                ps_sb[st : st + 32, cb : cb + 32],
                                lhsT=qt[h // 2][rb : rb + 64, 32 * sl : 32 * sl + 32],
                                rhs=kt[h // 2][rb : rb + 64, 32 * sl : 32 * sl + 32],
                                start=True,
                                stop=True,
                                tile_position=(rb, st),
                            )
                    # softmax over s (free dim), batched 64 problems per op
                    nc.scalar.activation(out=a_f[sb], in_=ps_sb, func=AF.Exp)
                    nc.vector.tensor_reduce(
                        out=sums[:, 16 * sb : 16 * sb + 16],
                        in_=a_f[sb].rearrange("p (j s) -> p j s", s=32),
                        axis=AX.X,
                        op=ALU.add,
                    )
                    rs_sl = rs[:, 16 * sb : 16 * sb + 16]
                    nc.vector.reciprocal(rs_sl, sums[:, 16 * sb : 16 * sb + 16])
                    rs_b = bass.AP(
                        tensor=rs_sl.tensor,
                        offset=rs_sl.offset,
                        ap=[*rs_sl.ap, [0, 32]],
                    )
                    nc.gpsimd.tensor_mul(
                        a_n[sb].rearrange("p (j s) -> p j s", s=32),
                        a_f[sb].rearrange("p (j s) -> p j s", s=32),
                        rs_b,
                    )
                    nc.vector.transpose(a_t[sb], a_n[sb])

            def av_group(vt, a_t, evac_fn, grp):
                """One bank-pair of AV matmuls + evacuation (pure casts).

                psum bank r = row tile 32r holds the 32 problems with
                sl%4 == r; partition rows 64*(h%2) = head parity.  The a_t
                input block sits at the scores layout column
                cb_s = 32*((h//2)*4 + sl//4); the OUTPUT goes to column
                cc = 8j + 4a + 2q4 + i (hh = h//2 = 2j+i, sl//4 = 2a+q4) so
                that (i,g) merges into one AP dim for the fp8 evacuation.
                Banks {0,1} fill and drain before {2,3} allocate, smoothing
                psum-ring pressure.
                """
                ys = [pp.tile([128, CH], f32, tag="ps", name="ps_y") for _ in range(2)]
                for sb in range(2):
                    for sl in range(NSEQ):
                        if (sl % 4) // 2 != grp:
                            continue
                        st = 32 * (sl % 4)
                        s4 = sl // 4
                        a, q4 = s4 // 2, s4 % 2
                        for h in range(sb, H, 2):
                            cb_s = 32 * ((h // 2) * 4 + s4)
                            hh = h // 2
                            cc = 8 * (hh // 2) + 4 * a + 2 * q4 + (hh % 2)
                            nc.tensor.matmul(
                                ys[(sl % 4) % 2][
                                    64 * sb : 64 * sb + 64, 32 * cc : 32 * cc + 32
                                ],
                                lhsT=vt[sl // 4][st : st + 32, 64 * h : 64 * h + 64],
                                rhs=a_t[sb][st : st + 32, cb_s : cb_s + 32],
                                start=True,
                                stop=True,
                                tile_position=(st, 64 * sb),
                            )
                for rr in range(2):
                    evac_fn(2 * grp + rr, ys[rr])

            def produce_lin(i):
                """Linears for pipeline step i (i<16: stage-1 chunk i,
                i>=16: stage-2 chunk i-16).  Returns the attention inputs."""
                qt = [sbqk.tile([128, CH], bf16, tag=f"qt{m}", name=f"qt{m}") for m in range(4)]
                kt = [sbqk.tile([128, CH], bf16, tag=f"kt{m}", name=f"kt{m}") for m in range(4)]
                vt = [sbv.tile([128, D], bf16, tag=f"vt{j}", name=f"vt{j}") for j in range(4)]
                if i < NCHUNK:
                    c = i
                    cols = slice(CH * c, CH * (c + 1))
                    xc8 = []
                    for j in range(2):
                        t_ = sbx.tile([128, 2 * CH], e4, tag=f"xc8_{j}", name=f"xc8_{j}")
                        nc.sync.dma_start(
                            out=t_.rearrange("p (i t) -> p i t", i=2),
                            in_=x8_d[:, 2 * j : 2 * j + 2, cols],
                        )
                        xc8.append(t_.rearrange("p (i t) -> p i t", i=2))
                    rhs8 = [xc8[j][:, :, :] for j in range(2)]
                    lhsT8 = [
                        [xc8[j][:, :, 128 * jj : 128 * (jj + 1)] for j in range(2)]
                        for jj in range(4)
                    ]
                    eq = lambda: linear_fp8(wt8["wq1"], bt["bq1"], rhs8, qt)
                    ek = lambda: linear_fp8(wt8["wk1"], None, rhs8, kt)
                    ev = lambda: linear_fp8_token_major(wt8["wv1"], lhsT8, vt)

                    def evac_fn(r, ys_r, c=c):
                        # 2 ops per bank: pass j -> [p, a, q4, (i,g)]
                        for j in range(2):
                            src = ys_r[:, 256 * j : 256 * j + 256].rearrange(
                                "p (a q ig) -> p a q ig", a=2, q=2
                            )
                            dst = y1s[j][:, r, 2 * c : 2 * c + 2, :, :]
                            if (r + j) % 2:
                                nc.scalar.copy(dst, src)
                            else:
                                nc.vector.tensor_copy(dst, src)

                    return (eq, ek, ev), dict(qt=qt, kt=kt, vt=vt, evac=evac_fn, y2t=None)
                c2 = i - NCHUNK
                cols = slice(CH * c2, CH * (c2 + 1))
                rhs8 = [y1r[j][:, :, cols] for j in range(2)]
                lhsT8 = [
                    [
                        y1r[j][:, :, CH * c2 + 128 * jj : CH * c2 + 128 * (jj + 1)]
                        for j in range(2)
                    ]
                    for jj in range(4)
                ]
                eq = lambda: linear_fp8(wt8["wq2"], bt["bq2"], rhs8, qt)
                ek = lambda: linear_fp8(wt8["wk2"], None, rhs8, kt)
                ev = lambda: linear_fp8_token_major(wt8["wv2"], lhsT8, vt)
                y2t = [sbo.tile([128, CH], bf16, tag=f"y2_{r}", name=f"y2_{r}") for r in range(4)]
                y2x = [
                    y2t[hh].rearrange(
                        "p (a q4 qr g) -> p qr a q4 g", a=2, q4=2, qr=4, g=32
                    )
                    for hh in range(4)
                ]

                def evac_fn(r, ys_r):
                    ysr = ys_r.rearrange(
                        "p (jj a q4 ii g) -> p jj ii a q4 g", jj=2, a=2, q4=2, ii=2
                    )
                    for hh in range(4):
                        src = ysr[:, hh // 2, hh % 2]
                        dst = y2x[hh][:, r, :, :, :]
                        if (r + hh) % 2 == 0:
                            nc.vector.tensor_copy(dst, src)
                        else:
                            nc.scalar.copy(dst, src)

                return (eq, ek, ev), dict(qt=qt, kt=kt, vt=vt, evac=evac_fn, y2t=y2t)

            def out_linear(c2, y2t, ms=(0, 1, 2, 3)):
                for m in ms:
                    ps = pp.tile([128, CH], f32, tag="ps", name="ps_lin")
                    for k in range(4):
                        nc.tensor.matmul(
                            ps,
                            lhsT=wo_t[k][:, 128 * m : 128 * (m + 1)],
                            rhs=y2t[k],
                            start=(k == 0),
                            stop=(k == 3),
                        )
                    os_ = sbo.tile([128, CH], bf16, tag=f"os{m}", name=f"os{m}")
                    if m % 2 == 0:
                        nc.scalar.activation(
                            out=os_, in_=ps, func=AF.Identity,
                            bias=bt["bo"][:, m : m + 1],
                        )
                    else:
                        nc.vector.tensor_scalar_add(os_, ps, bt["bo"][:, m : m + 1])
                    nc.sync.dma_start(
                        out=out_d[128 * m : 128 * (m + 1), CH * c2 : CH * (c2 + 1)],
                        in_=os_,
                    )

            # ---- software pipeline over 32 steps (16 per stage): the PE
            # stream per step interleaves the low-duty attention matmuls
            # with the dense fp8 bursts: [scores-half0(i), q-lin(i+1),
            # scores-half1(i), k-lin(i+1), outlin(i-1), v-lin(i+1), AV(i)]
            # — power smoothing plus extra psum drain slack.  Exception:
            # stage-2 linears read y1, so produce_lin(16) comes after
            # av_evac(15).
            NSTEP = 2 * NCHUNK
            emits, st = produce_lin(0)
            for e in emits:
                e()
            # remaining (stage-2 / out) weights load behind the critical path
            for name in ("wq2", "wk2", "wv2"):
                load_w8(name)
            load_wo()
            for name in ("bq2", "bo"):
                load_b(name)
            pend_out = None
            for i in range(NSTEP):
                sst = scores_begin()
                scores_half(st["qt"], st["kt"], 0, sst)
                if i + 1 < NSTEP and i != NCHUNK - 1:
                    (eq, ek, ev), st_next = produce_lin(i + 1)
                    eq()
                    scores_half(st["qt"], st["kt"], 1, sst)
                    ek()
                    ev()
                else:
                    st_next = None
                    scores_half(st["qt"], st["kt"], 1, sst)
                av_group(st["vt"], sst["a_t"], st["evac"], 0)
                if pend_out is not None:
                    out_linear(*pend_out, ms=(0, 1))
                av_group(st["vt"], sst["a_t"], st["evac"], 1)
                if pend_out is not None:
                    out_linear(*pend_out, ms=(2, 3))
                    pend_out = None
                if i == NCHUNK - 1:
                    (eq, ek, ev), st_next = produce_lin(i + 1)
                    eq()
                    ek()
                    ev()
                if st["y2t"] is not None:
                    pend_out = (i - NCHUNK, st["y2t"])
                st = st_next
            out_linear(*pend_out)
    nc.finalize()
    return nc


def _get_graph():
    global _GRAPH
    if _GRAPH is None:
        _GRAPH = _build_graph()
    return _GRAPH


def _host_pack(x, q1_w, q1_b, k1_w, k1_b, v1_w, v1_b, q2_w, q2_b, k2_w, k2_b,
               v2_w, v2_b, out_w, out_b):
    import ml_dtypes

    bf = ml_dtypes.bfloat16
    e4 = ml_dtypes.float8_e4m3
    scale = 1.0 / np.sqrt(E)

    def w8(w, s=1.0):
        # [128, 4, D]: w8[p, 2j+i, o] = (W*s)[o, 256j+128i+p]
        wT = (w * s).astype(np.float32).T.reshape(2, 2, 128, D)  # [j, i, p, o]
        return np.ascontiguousarray(wT.transpose(2, 0, 1, 3).reshape(128, 4, D)).astype(e4)

    def bia(b, s=1.0):
        return np.ascontiguousarray(
            (np.asarray(b, dtype=np.float64) * s).astype(np.float32).reshape(4, 128).T
        )

    # fold v-biases into downstream biases (softmax rows sum to 1):
    #   y1_true = y1_nb + 1 v1_b^T  =>  q2/v2 biases pick up W @ v1_b
    #   y2_true = y2_nb + 1 c2^T, c2 = v2_b + v2_w@v1_b  =>  out bias += out_w@c2
    # k-biases are dropped (softmax shift invariance).
    q1_bf = np.asarray(q1_b, dtype=np.float64)
    v1_bf = np.asarray(v1_b, dtype=np.float64)
    q2_be = np.asarray(q2_b, dtype=np.float64) + np.asarray(q2_w, dtype=np.float64) @ v1_bf
    c2 = np.asarray(v2_b, dtype=np.float64) + np.asarray(v2_w, dtype=np.float64) @ v1_bf
    bo_e = np.asarray(out_b, dtype=np.float64) + np.asarray(out_w, dtype=np.float64) @ c2

    common = {
        "wq1": w8(q1_w, scale), "wk1": w8(k1_w), "wv1": w8(v1_w),
        "wq2": w8(q2_w, scale), "wk2": w8(k2_w), "wv2": w8(v2_w),
        "wo": np.ascontiguousarray(np.asarray(out_w, dtype=np.float32).T).astype(bf),
        "bq1": bia(q1_bf, scale),
        "bq2": bia(q2_be, scale),
        "bo": bia(bo_e),
    }
    in_maps = []
    for b in range(B):
        # x[b]: [1024(ch=ng*32+gg), 8(p), 512] -> token order t = ng*256+p*32+gg
        xb = np.asarray(x[b]).reshape(NUM_G, G, NUM_P, D)
        xb = xb.transpose(0, 2, 1, 3).reshape(T, D)
        # x8[p, 2j+i, t] = xb[t, 256j+128i+p]
        x8 = xb.T.reshape(2, 2, 128, T).transpose(2, 0, 1, 3).reshape(128, 4, T)
        m = dict(common)
        m["x8"] = np.ascontiguousarray(x8).astype(e4)
        in_maps.append(m)
    return in_maps


def _host_unpack(results):
    # device out: [512, 8192] f32, cols packed:
    # col(ng,gg,p) = (gg//2)*512 + (gg%2)*256 + p*32 + ng
    ng_, gg_, p_ = np.meshgrid(
        np.arange(NUM_G), np.arange(G), np.arange(NUM_P), indexing="ij"
    )
    idx = (gg_ // 2) * 512 + (gg_ % 2) * 256 + p_ * 32 + ng_
    out = np.empty((B, NUM_G * G, NUM_P, D), dtype=np.float32)
    for b in range(B):
        y = results[b]["out"].T.astype(np.float32)  # [8192, 512]
        out[b] = y[idx].reshape(NUM_G * G, NUM_P, D)
    return out


def kernel(**inputs):
    global LAST_EXEC_TIME_NS, LAST_TRACE
    from concourse.bass_utils import run_bass_kernel_spmd

    nc = _get_graph()
    in_maps = _host_pack(**inputs)
    trace = os.environ.get("KBENCH_TRACE") == "1"
    res = run_bass_kernel_spmd(nc, in_maps, list(range(8)), trace=trace)
    LAST_EXEC_TIME_NS = res.exec_time_ns
    it = res.instructions_and_trace
    LAST_TRACE = it[1] if it else None
    return _host_unpack(res.results)


# revision 45
# speedup vs baseline: 1.1080x; 1.0022x over previous
"""AsymAttentionLayer Trainium2 kernel — data-parallel over B on 8 NeuronCores.

Reference computation (per batch element b, NUM_G=32, g=32, num_p=8, dim=512,
H=8, E=64):
  stage1: attention within groups of g=32 tokens (seq=(ng,p), pos=gg)
  stage2: attention across groups (seq=(gg,p), pos=ng)
  7 linears of [tokens,512]x[512,512].

v2: the six q/k/v linears run in fp8e4m3 with MatmulPerfMode.DoubleRow
(256-deep contraction per pass, 2x the bf16 column rate); the out linear
stays bf16 (its quantization error alone would exceed the error budget).
All v-biases are folded into downstream biases on the host (softmax rows
sum to 1, so A@(V+1 b^T) = A@V + 1 b^T), which turns every attention-output
evacuation into a pure cast that can run on any engine.  Attention math is
unchanged from the baseline (bf16, PE-quadrant-packed 32x32 blocks).

Device-side layouts: fp8 activations/weights use the DoubleRow pairing
[128 part, (i in 2, col)] where partition p + pair i covers input feature
256*j + 128*i + p for pass j.  Token order t = ng*256 + p*32 + gg.
"""

import os
import sys

import numpy as np

sys.path.insert(0, "/opt/trn_rl_repo")

NUM_G = 32
G = 32
NUM_P = 8
B = 8
D = 512
H = 8
E = 64
T = NUM_G * G * NUM_P  # 8192 tokens per core
CH = 512  # tokens per chunk
NCHUNK = T // CH  # 16
NSEQ = CH // G  # 16 sequences per chunk

_GRAPH = None
LAST_EXEC_TIME_NS = None
LAST_TRACE = None


def _build_graph():
    import concourse.bass as bass
    from concourse import bacc, mybir
    from concourse.tile import TileContext

    f32 = mybir.dt.float32
    bf16 = mybir.dt.bfloat16
    e4 = mybir.dt.float8e4
    DR = mybir.MatmulPerfMode.DoubleRow
    AF = mybir.ActivationFunctionType
    ALU = mybir.AluOpType
    AX = mybir.AxisListType

    nc = bacc.Bacc()

    # fp8 activations, DoubleRow-paired: x8[p, 2j+i, t] = x[t, 256j+128i+p]
    x8_d = nc.declare_dram_parameter("x8", [128, 4, T], e4, isOutput=False)
    w8_d = {}
    for name in ("wq1", "wk1", "wv1", "wq2", "wk2", "wv2"):
        # w8[p, 2j+i, o] = W'[o, 256j+128i+p]
        w8_d[name] = nc.declare_dram_parameter(name, [128, 4, D], e4, isOutput=False)
    wo_d = nc.declare_dram_parameter("wo", [D, D], bf16, isOutput=False)
    b_d = {}
    for name in ("bq1", "bq2", "bo"):
        # k-biases are dropped entirely: softmax over s is invariant to the
        # s-independent terms q_l.bk + bq.bk, so S' = (q+bq).k is exact.
        b_d[name] = nc.declare_dram_parameter(name, [128, 4], f32, isOutput=False)
    out_d = nc.declare_dram_parameter("out", [D, T], bf16, isOutput=True)

    with TileContext(nc) as tc:
        with (
            tc.tile_pool(name="wpool", bufs=1) as wpool,
            tc.tile_pool(name="bpool", bufs=1) as bpool,
            tc.tile_pool(name="y1pool", bufs=1) as y1pool,
            tc.tile_pool(name="sbx", bufs=4) as sbx,
            tc.tile_pool(name="sbqk", bufs=2) as sbqk,
            tc.tile_pool(name="sbv", bufs=2) as sbv,
            tc.tile_pool(name="sba", bufs=2) as sba,
            tc.tile_pool(name="sbo", bufs=2) as sbo,
            tc.tile_pool(name="pp", bufs=8, space="PSUM") as pp,
        ):
            # ---- weights / biases resident in SBUF ----
            wt8 = {}
            wo_t = []
            bt = {}

            def load_w8(name):
                t_ = wpool.tile([128, 4 * D], e4, tag=f"w8_{name}", name=f"w8_{name}")
                nc.sync.dma_start(
                    out=t_.rearrange("p (a f) -> p a f", a=4), in_=w8_d[name][:, :, :]
                )
                # [p, j, i, f]
                wt8[name] = t_.rearrange("p (j i f) -> p j i f", j=2, i=2)

            def load_wo():
                for k in range(4):
                    t_ = wpool.tile([128, D], bf16, tag=f"wo_{k}", name=f"wo_{k}")
                    nc.sync.dma_start(out=t_, in_=wo_d[128 * k : 128 * (k + 1), :])
                    wo_t.append(t_)

            def load_b(name):
                t_ = bpool.tile([128, 4], f32, tag=name, name=f"b_{name}")
                nc.sync.dma_start(out=t_, in_=b_d[name][:, :])
                bt[name] = t_

            for name in ("wq1", "wk1", "wv1"):
                load_w8(name)
            load_b("bq1")

            # stage-1 output, fp8 DoubleRow-paired: y1_8[j] cols = i*T + t
            y1_8 = [
                y1pool.tile([128, 2 * T], e4, tag=f"y18_{j}", name=f"y18_{j}")
                for j in range(2)
            ]
            y1r = [y1_8[j].rearrange("p (i t) -> p i t", i=2) for j in range(2)]
            # scatter view for av_evac with merged (i,g) dim:
            # col = (i*32+g)*256 + q4*128 + r*32 + ng
            y1s = [
                y1_8[j].rearrange(
                    "p (ig q4 qr n) -> p qr n q4 ig", ig=64, q4=2, qr=4, n=32
                )
                for j in range(2)
            ]

            def linear_fp8(w8r, bias_tile, rhs8, out_tiles):
                """Feature-major fp8 DoubleRow linear: out[m] = W'^T@x (+ b).

                rhs8: list of 2 APs [128, 2, 512] (pass j in {0,1}).
                out_tiles: 4 SBUF tiles [128, 512] bf16.
                Evacuations alternate ACT/DVE so two drains run in parallel
                with the PE's psum production.
                """
                for m in range(4):
                    ps = pp.tile([128, CH], f32, tag="ps", name="ps_lin")
                    for j in range(2):
                        nc.tensor.matmul(
                            ps,
                            lhsT=w8r[:, j, :, 128 * m : 128 * (m + 1)],
                            rhs=rhs8[j],
                            start=(j == 0),
                            stop=(j == 1),
                            perf_mode=DR,
                        )
                    if bias_tile is None:
                        if m % 2 == 0:
                            nc.vector.tensor_copy(out_tiles[m], ps)
                        else:
                            nc.scalar.copy(out_tiles[m], ps)
                    elif m % 2 == 0:
                        nc.scalar.activation(
                            out=out_tiles[m],
                            in_=ps,
                            func=AF.Identity,
                            bias=bias_tile[:, m : m + 1],
                        )
                    else:
                        nc.vector.tensor_scalar_add(
                            out_tiles[m], ps, bias_tile[:, m : m + 1]
                        )

            def linear_fp8_token_major(w8r, lhsT8, out_tiles):
                """v-linear: out[j] = [128 tokens, 512 dims], no bias (folded).

                lhsT8[jj][j]: AP [128, 2, 128] - token group jj, pass j.
                """
                for jj in range(4):
                    ps = pp.tile([128, D], f32, tag="ps", name="ps_vlin")
                    for j in range(2):
                        nc.tensor.matmul(
                            ps,
                            lhsT=lhsT8[jj][j],
                            rhs=w8r[:, j],
                            start=(j == 0),
                            stop=(j == 1),
                            perf_mode=DR,
                        )
                    if jj % 2 == 0:
                        nc.vector.tensor_copy(out_tiles[jj], ps)
                    else:
                        nc.scalar.copy(out_tiles[jj], ps)

            def scores_begin():
                a_f = [sba.tile([128, CH], bf16, tag=f"a{i}", name=f"a_f{i}") for i in range(2)]
                sums = sba.tile([128, 32], f32, tag="sums")
                a_n = [sba.tile([128, CH], bf16, tag=f"an{i}", name=f"a_n{i}") for i in range(2)]
                a_t = [sba.tile([128, CH], bf16, tag=f"at{i}", name=f"a_t{i}") for i in range(2)]
                rs = sba.tile([128, 32], f32, tag="rs")
                return dict(a_f=a_f, sums=sums, a_n=a_n, a_t=a_t, rs=rs)

            def scores_half(qt, kt, sb, sst):
                """One head-parity's scores + softmax (bank sb).  Emitted in
                two halves so the fp8 linear bursts interleave with the
                low-duty attention matmuls (power smoothing + psum slack).

                PSUM bank index always equals the PE row-tile index so that
                concurrently-running row tiles never write the same bank.
                scores placement: bank=h%2 (= row tile 64*(h%2)), strip=sl%4,
                colblk=(h//2)*4 + sl//4.
                """
                a_f, sums, a_n, a_t, rs = (
                    sst["a_f"], sst["sums"], sst["a_n"], sst["a_t"], sst["rs"]
                )
                if True:
                    ps_sb = pp.tile([128, CH], f32, tag="ps", name="ps_s")
                    for sl in range(NSEQ):
                        st = 32 * (sl % 4)
                        for h in range(sb, H, 2):
                            cb = 32 * ((h // 2) * 4 + sl // 4)
                            rb = 64 * sb
                            nc.tensor.matmul(
                                ps_sb[st : st + 32, cb : cb + 32],
                                lhsT=qt[h // 2][rb : rb + 64, 32 * sl : 32 * sl + 32],
                                rhs=kt[h // 2][rb : rb + 64, 32 * sl : 32 * sl + 32],
                                start=True,
                                stop=True,
                                tile_position=(rb, st),
                            )
                    # softmax over s (free dim), batched 64 problems per op
                    nc.scalar.activation(out=a_f[sb], in_=ps_sb, func=AF.Exp)
                    nc.vector.tensor_reduce(
                        out=sums[:, 16 * sb : 16 * sb + 16],
                        in_=a_f[sb].rearrange("p (j s) -> p j s", s=32),
                        axis=AX.X,
                        op=ALU.add,
                    )
                    rs_sl = rs[:, 16 * sb : 16 * sb + 16]
                    nc.vector.reciprocal(rs_sl, sums[:, 16 * sb : 16 * sb + 16])
                    rs_b = bass.AP(
                        tensor=rs_sl.tensor,
                        offset=rs_sl.offset,
                        ap=[*rs_sl.ap, [0, 32]],
                    )
                    nc.gpsimd.tensor_mul(
                        a_n[sb].rearrange("p (j s) -> p j s", s=32),
                        a_f[sb].rearrange("p (j s) -> p j s", s=32),
                        rs_b,
                    )
                    nc.vector.transpose(a_t[sb], a_n[sb])

            def av_group(vt, a_t, evac_fn, grp):
                """One bank-pair of AV matmuls + evacuation (pure casts).

                psum bank r = row tile 32r holds the 32 problems with
                sl%4 == r; partition rows 64*(h%2) = head parity.  The a_t
                input block sits at the scores layout column
                cb_s = 32*((h//2)*4 + sl//4); the OUTPUT goes to column
                cc = 8j + 4a + 2q4 + i (hh = h//2 = 2j+i, sl//4 = 2a+q4) so
                that (i,g) merges into one AP dim for the fp8 evacuation.
                Banks {0,1} fill and drain before {2,3} allocate, smoothing
                psum-ring pressure.
                """
                ys = [pp.tile([128, CH], f32, tag="ps", name="ps_y") for _ in range(2)]
                for sb in range(2):
                    for sl in range(NSEQ):
                        if (sl % 4) // 2 != grp:
                            continue
                        st = 32 * (sl % 4)
                        s4 = sl // 4
                        a, q4 = s4 // 2, s4 % 2
                        for h in range(sb, H, 2):
                            cb_s = 32 * ((h // 2) * 4 + s4)
                            hh = h // 2
                            cc = 8 * (hh // 2) + 4 * a + 2 * q4 + (hh % 2)
                            nc.tensor.matmul(
                                ys[(sl % 4) % 2][
                                    64 * sb : 64 * sb + 64, 32 * cc : 32 * cc + 32
                                ],
                                lhsT=vt[sl // 4][st : st + 32, 64 * h : 64 * h + 64],
                                rhs=a_t[sb][st : st + 32, cb_s : cb_s + 32],
                                start=True,
                                stop=True,
                                tile_position=(st, 64 * sb),
                            )
                for rr in range(2):
                    evac_fn(2 * grp + rr, ys[rr])

            xq = {}

            def issue_x_dma(c):
                """Prefetch chunk c's fp8 activations (depth-3 pipelined)."""
                if not (0 <= c < NCHUNK) or c in xq:
                    return
                cols = slice(CH * c, CH * (c + 1))
                xc8 = []
                for j in range(2):
                    t_ = sbx.tile([128, 2 * CH], e4, tag=f"xc8_{j}", name=f"xc8_{j}")
                    nc.sync.dma_start(
                        out=t_.rearrange("p (i t) -> p i t", i=2),
                        in_=x8_d[:, 2 * j : 2 * j + 2, cols],
                    )
                    xc8.append(t_.rearrange("p (i t) -> p i t", i=2))
                xq[c] = xc8

            def produce_lin(i):
                """Linears for pipeline step i (i<16: stage-1 chunk i,
                i>=16: stage-2 chunk i-16).  Returns the attention inputs."""
                qt = [sbqk.tile([128, CH], bf16, tag=f"qt{m}", name=f"qt{m}") for m in range(4)]
                kt = [sbqk.tile([128, CH], bf16, tag=f"kt{m}", name=f"kt{m}") for m in range(4)]
                vt = [sbv.tile([128, D], bf16, tag=f"vt{j}", name=f"vt{j}") for j in range(4)]
                if i < NCHUNK:
                    c = i
                    issue_x_dma(c)
                    xc8 = xq.pop(c)
                    issue_x_dma(c + 3)
                    rhs8 = [xc8[j][:, :, :] for j in range(2)]
                    lhsT8 = [
                        [xc8[j][:, :, 128 * jj : 128 * (jj + 1)] for j in range(2)]
                        for jj in range(4)
                    ]
                    eq = lambda: linear_fp8(wt8["wq1"], bt["bq1"], rhs8, qt)
                    ek = lambda: linear_fp8(wt8["wk1"], None, rhs8, kt)
                    ev = lambda: linear_fp8_token_major(wt8["wv1"], lhsT8, vt)

                    def evac_fn(r, ys_r, c=c):
                        # 2 ops per bank: pass j -> [p, a, q4, (i,g)]
                        for j in range(2):
                            src = ys_r[:, 256 * j : 256 * j + 256].rearrange(
                                "p (a q ig) -> p a q ig", a=2, q=2
                            )
                            dst = y1s[j][:, r, 2 * c : 2 * c + 2, :, :]
                            if (r + j) % 2:
                                nc.scalar.copy(dst, src)
                            else:
                                nc.vector.tensor_copy(dst, src)

                    return (eq, ek, ev), dict(qt=qt, kt=kt, vt=vt, evac=evac_fn, y2t=None)
                c2 = i - NCHUNK
                cols = slice(CH * c2, CH * (c2 + 1))
                rhs8 = [y1r[j][:, :, cols] for j in range(2)]
                lhsT8 = [
                    [
                        y1r[j][:, :, CH * c2 + 128 * jj : CH * c2 + 128 * (jj + 1)]
                        for j in range(2)
                    ]
                    for jj in range(4)
                ]
                eq = lambda: linear_fp8(wt8["wq2"], bt["bq2"], rhs8, qt)
                ek = lambda: linear_fp8(wt8["wk2"], None, rhs8, kt)
                ev = lambda: linear_fp8_token_major(wt8["wv2"], lhsT8, vt)
                y2t = [sbo.tile([128, CH], bf16, tag=f"y2_{r}", name=f"y2_{r}") for r in range(4)]
                y2x = [
                    y2t[hh].rearrange(
                        "p (a q4 qr g) -> p qr a q4 g", a=2, q4=2, qr=4, g=32
                    )
                    for hh in range(4)
                ]

                def evac_fn(r, ys_r):
                    ysr = ys_r.rearrange(
                        "p (jj a q4 ii g) -> p jj ii a q4 g", jj=2, a=2, q4=2, ii=2
                    )
                    for hh in range(4):
                        src = ysr[:, hh // 2, hh % 2]
                        dst = y2x[hh][:, r, :, :, :]
                        if (r + hh) % 2 == 0:
                            nc.vector.tensor_copy(dst, src)
                        else:
                            nc.scalar.copy(dst, src)

                return (eq, ek, ev), dict(qt=qt, kt=kt, vt=vt, evac=evac_fn, y2t=y2t)

            def out_linear(c2, y2t, ms=(0, 1, 2, 3)):
                for m in ms:
                    ps = pp.tile([128, CH], f32, tag="ps", name="ps_lin")
                    for k in range(4):
                        nc.tensor.matmul(
                            ps,
                            lhsT=wo_t[k][:, 128 * m : 128 * (m + 1)],
                            rhs=y2t[k],
                            start=(k == 0),
                            stop=(k == 3),
                        )
                    os_ = sbo.tile([128, CH], bf16, tag=f"os{m}", name=f"os{m}")
                    if m % 2 == 0:
                        nc.scalar.activation(
                            out=os_, in_=ps, func=AF.Identity,
                            bias=bt["bo"][:, m : m + 1],
                        )
                    else:
                        nc.vector.tensor_scalar_add(os_, ps, bt["bo"][:, m : m + 1])
                    nc.sync.dma_start(
                        out=out_d[128 * m : 128 * (m + 1), CH * c2 : CH * (c2 + 1)],
                        in_=os_,
                    )

            # ---- software pipeline over 32 steps (16 per stage): the PE
            # stream per step interleaves the low-duty attention matmuls
            # with the dense fp8 bursts: [scores-half0(i), q-lin(i+1),
            # scores-half1(i), k-lin(i+1), outlin(i-1), v-lin(i+1), AV(i)]
            # — power smoothing plus extra psum drain slack.  Exception:
            # stage-2 linears read y1, so produce_lin(16) comes after
            # av_evac(15).
            NSTEP = 2 * NCHUNK
            for c in range(3):
                issue_x_dma(c)
            emits, st = produce_lin(0)
            for e in emits:
                e()
            # remaining (stage-2 / out) weights load behind the critical path
            for name in ("wq2", "wk2", "wv2"):
                load_w8(name)
            load_wo()
            for name in ("bq2", "bo"):
                load_b(name)
            pend_out = None
            for i in range(NSTEP):
                sst = scores_begin()
                scores_half(st["qt"], st["kt"], 0, sst)
                if i + 1 < NSTEP and i != NCHUNK - 1:
                    (eq, ek, ev), st_next = produce_lin(i + 1)
                    eq()
                    scores_half(st["qt"], st["kt"], 1, sst)
                    ek()
                    ev()
                else:
                    st_next = None
                    if pend_out is not None:
                        out_linear(*pend_out, ms=(0, 1))
                    scores_half(st["qt"], st["kt"], 1, sst)
                    if pend_out is not None:
                        out_linear(*pend_out, ms=(2, 3))
                        pend_out = None
                av_group(st["vt"], sst["a_t"], st["evac"], 0)
                if pend_out is not None:
                    out_linear(*pend_out, ms=(0, 1))
                av_group(st["vt"], sst["a_t"], st["evac"], 1)
                if pend_out is not None:
                    out_linear(*pend_out, ms=(2, 3))
                    pend_out = None
                if i == NCHUNK - 1:
                    (eq, ek, ev), st_next = produce_lin(i + 1)
                    eq()
                    ek()
                    ev()
                if st["y2t"] is not None:
                    pend_out = (i - NCHUNK, st["y2t"])
                st = st_next
            out_linear(*pend_out)
    nc.finalize()
    return nc


def _get_graph():
    global _GRAPH
    if _GRAPH is None:
        _GRAPH = _build_graph()
    return _GRAPH


def _host_pack(x, q1_w, q1_b, k1_w, k1_b, v1_w, v1_b, q2_w, q2_b, k2_w, k2_b,
               v2_w, v2_b, out_w, out_b):
    import ml_dtypes

    bf = ml_dtypes.bfloat16
    e4 = ml_dtypes.float8_e4m3
    scale = 1.0 / np.sqrt(E)

    def w8(w, s=1.0):
        # [128, 4, D]: w8[p, 2j+i, o] = (W*s)[o, 256j+128i+p]
        wT = (w * s).astype(np.float32).T.reshape(2, 2, 128, D)  # [j, i, p, o]
        return np.ascontiguousarray(wT.transpose(2, 0, 1, 3).reshape(128, 4, D)).astype(e4)

    def bia(b, s=1.0):
        return np.ascontiguousarray(
            (np.asarray(b, dtype=np.float64) * s).astype(np.float32).reshape(4, 128).T
        )

    # fold v-biases into downstream biases (softmax rows sum to 1):
    #   y1_true = y1_nb + 1 v1_b^T  =>  q2/v2 biases pick up W @ v1_b
    #   y2_true = y2_nb + 1 c2^T, c2 = v2_b + v2_w@v1_b  =>  out bias += out_w@c2
    # k-biases are dropped (softmax shift invariance).
    q1_bf = np.asarray(q1_b, dtype=np.float64)
    v1_bf = np.asarray(v1_b, dtype=np.float64)
    q2_be = np.asarray(q2_b, dtype=np.float64) + np.asarray(q2_w, dtype=np.float64) @ v1_bf
    c2 = np.asarray(v2_b, dtype=np.float64) + np.asarray(v2_w, dtype=np.float64) @ v1_bf
    bo_e = np.asarray(out_b, dtype=np.float64) + np.asarray(out_w, dtype=np.float64) @ c2

    common = {
        "wq1": w8(q1_w, scale), "wk1": w8(k1_w), "wv1": w8(v1_w),
        "wq2": w8(q2_w, scale), "wk2": w8(k2_w), "wv2": w8(v2_w),
        "wo": np.ascontiguousarray(np.asarray(out_w, dtype=np.float32).T).astype(bf),
        "bq1": bia(q1_bf, scale),
        "bq2": bia(q2_be, scale),
        "bo": bia(bo_e),
    }
    in_maps = []
    for b in range(B):
        # x[b]: [1024(ch=ng*32+gg), 8(p), 512] -> token order t = ng*256+p*32+gg
        xb = np.asarray(x[b]).reshape(NUM_G, G, NUM_P, D)
        xb = xb.transpose(0, 2, 1, 3).reshape(T, D)
        # x8[p, 2j+i, t] = xb[t, 256j+128i+p]
        x8 = xb.T.reshape(2, 2, 128, T).transpose(2, 0, 1, 3).reshape(128, 4, T)
        m = dict(common)
        m["x8"] = np.ascontiguousarray(x8).astype(e4)
        in_maps.append(m)
    return in_maps


def _host_unpack(results):
    # device out: [512, 8192] f32, cols packed:
    # col(ng,gg,p) = (gg//2)*512 + (gg%2)*256 + p*32 + ng
    ng_, gg_, p_ = np.meshgrid(
        np.arange(NUM_G), np.arange(G), np.arange(NUM_P), indexing="ij"
    )
    idx = (gg_ // 2) * 512 + (gg_ % 2) * 256 + p_ * 32 + ng_
    out = np.empty((B, NUM_G * G, NUM_P, D), dtype=np.float32)
    for b in range(B):
        y = results[b]["out"].T.astype(np.float32)  # [8192, 512]
        out[b] = y[idx].reshape(NUM_G * G, NUM_P, D)
    return out


def kernel(**inputs):
    global LAST_EXEC_TIME_NS, LAST_TRACE
    from concourse.bass_utils import run_bass_kernel_spmd

    nc = _get_graph()
    in_maps = _host_pack(**inputs)
    trace = os.environ.get("KBENCH_TRACE") == "1"
    res = run_bass_kernel_spmd(nc, in_maps, list(range(8)), trace=trace)
    LAST_EXEC_TIME_NS = res.exec_time_ns
    it = res.instructions_and_trace
    LAST_TRACE = it[1] if it else None
    return _host_unpack(res.results)
